# revision 29
# baseline (speedup 1.0000x reference)
"""Trainium2 Bass kernel for nn_Decoder_72911364816952 (v2).

Strategy (8 NeuronCores, memory-bound regime):
  Phase A (all cores): each core reads a distinct 1/8 column-shard of the
    dynamic-weight generator matrices and computes the generated per-sample
    conv weights for all samples. The bt (surf) generator and the wave
    biases are stored fp8 (numerically validated); the rest bf16.
  AllToAll (2 per layer, bf16): routes every sample's generated weights to
    the core that runs that sample. Part 1 carries bt+biases+pass1 convs,
    part 2 the residual-block convs.
  Emission order is the key scheduling device (engine queues are in-order):
    phase A of layers 0 AND 1 is emitted before any phase-B work, so the
    first A2A's global-barrier skew is absorbed by layer-1 generation work;
    layer 2's phase A is drained in small pieces inside phase B of layers
    0/1. Weight loads ride the scalar queue (dependency-aligned with the
    activations that need them), contrib stores + collectives on gpsimd,
    input streaming on sync.
  Phase B (each core, one sample): spatial decoder entirely on-chip with a
    single f16 activation accumulator (validated): fused Sin surf path,
    Sobel as f16 shifted adds + folded channel matmuls, instance norm folded
    into the next conv's weights, residual block in bf16 with Prelu lrelu,
    bilinear-2x + gauss as f16 vector ops.
  Output: each core writes its sample's (3,64,64) image; host stacks 0-3.
"""
import math
import os
import sys

sys.path.insert(0, "/opt/trn_rl_repo")

import numpy as np

# ---------------------------------------------------------------- constants
L = 3
LAT = 256
F = 64
CPE = 32
B = 4
NCORE = 8
SIZES = [16, 32, 64]
HWS = [s * s for s in SIZES]
K_FREQ = 8

S_BT = 512.0           # host scale on bt_W (fp8)
S_WB = 256.0           # host scale on wave biases (fp8)
SIN_SCALE = 1.0 / (S_BT * S_WB)

# per-rank wwsb layout: every rank holds a 1/8 fout-slice of every conv
P1_CONVS = [("c2w", 64, 32), ("w2cA", 32, 128), ("w2cB", 128, 128),
            ("g2c", 128, 128)]
P2_CONVS = [("skip", 256, 64), ("c1", 256, 128), ("c2", 128, 128),
            ("c3", 128, 64)]
BIAS_KEYS = [("bt", "bt_b", 64), ("c2w", "c2w_b", 32),
             ("skip", "cc_skip_b", 64), ("c1", "cc_c1_b", 128),
             ("c2", "cc_c2_b", 128), ("c3", "cc_c3_b", 64)]
BIAS_CUM = {}
_b = 0
for _n, _k, _fo in BIAS_KEYS:
    BIAS_CUM[_n] = _b
    _b += _fo

WSB_OFF = {}
_o = 0
for _n, _fi, _fo in P1_CONVS + P2_CONVS:
    _rows = min(_fi, 128)
    for _kc in range(_fi // _rows):
        WSB_OFF[(_n, _kc)] = _o
        _o += _rows * (_fo // NCORE)
P1B = WSB_OFF[("skip", 0)]         # 4924: wwsb part-1 width
WSB_USED = _o                      # 14140
WSB_W = 14336                      # padded to 2048-multiple
BT_W = 8192
CON_W = BT_W + WSB_W               # 22528
P1_CON = BT_W + P1B                # 13116 (contrib part-1 width)
P2_CON = CON_W - P1_CON
G = 2048                           # phase-A matmul group width
NGROUP = CON_W // G                # 11
E1 = math.e
_ga_raw = math.exp(-0.5)
GA = _ga_raw / (1.0 + 2.0 * _ga_raw)
GRBA = 1.0 / _ga_raw
UPGAUSS_C = 0.5625 * GA * GA
PI_2 = math.pi / 2.0
RSQRT_MAGIC = 0x5F3759DF
SIM_COMPAT = bool(os.environ.get("BASS_SIM_COMPAT"))


def pos_enc_np(size):
    p = np.arange(size, dtype=np.float32)
    feats = []
    for k in range(K_FREQ):
        ang = p * (2.0 ** k) * (2.0 * np.pi / size)
        s, c = np.sin(ang).astype(np.float32), np.cos(ang).astype(np.float32)
        feats += [np.broadcast_to(s[:, None], (size, size)),
                  np.broadcast_to(c[:, None], (size, size)),
                  np.broadcast_to(s[None, :], (size, size)),
                  np.broadcast_to(c[None, :], (size, size))]
    return np.ascontiguousarray(np.stack(feats).reshape(4 * K_FREQ, size * size))


INPUT_SHAPES = [
    ("wws8", (L, 128, 2 * BT_W), "f8"),
    ("wwsb", (L, LAT, WSB_W), "bf16"),
    ("latn", (L, LAT, 2 * B), "f32"),
    ("latnme", (L, LAT, 1), "f32"),
    ("biasW", (L, LAT, 480), "bf16"),
    ("leak", (1, 1), "f32"),
    ("seed", (F, 256), "f32"),
    ("wb0", (128, 8 * HWS[0]), "f8"),
    ("wb1", (128, 8 * HWS[1]), "f8"),
    ("wb2", (128, 8 * HWS[2]), "f8"),
    ("cw0", (CPE, HWS[0]), "f16"),
    ("cw1", (CPE, HWS[1]), "f16"),
    ("cw2", (CPE, HWS[2]), "f16"),
    ("imgWT", (F, 3), "f32"),
    ("imgb", (3, 1), "f32"),
]


# ---------------------------------------------------------------- device code
def build_kernel(tc, ins, out_img):
    import concourse.bass as bass
    from concourse import mybir

    nc = tc.nc
    f32 = mybir.dt.float32
    i32 = mybir.dt.int32
    bf16 = mybir.dt.bfloat16
    f16 = mybir.dt.float16
    f8 = mybir.dt.float8e4
    ALU = mybir.AluOpType
    ACTF = mybir.ActivationFunctionType
    AX = mybir.AxisListType
    ctxs = []

    def pool(name, bufs, space="SBUF"):
        p = tc.tile_pool(name=name, bufs=bufs, space=space)
        ctxs.append(p)
        return p.__enter__()

    def mk(pool_, shape, tag, dt=None, bufs=None, **kw):
        return pool_.tile(shape, dt or f32, name=tag, tag=tag, bufs=bufs, **kw)

    dram = pool("dram", 1, "DRAM")
    psp = pool("psp", 1, "PSUM")
    big = pool("big", 1)
    gw = pool("gw", 1)
    ab = pool("ab", 3)
    sm = pool("sm", 1)
    smc = pool("smc", 2)
    pha = pool("pha", 1)
    stg = pool("stg", 2)

    def mmr(ps, lhsT, rhs, start, stop):
        nc.tensor.matmul(ps, lhsT, rhs, start=start, stop=stop)

    # ---------------- setup: lat_new loads (host-computed), leak
    leak_sb = mk(sm, [1, 1], "leak")
    nc.sync.dma_start(leak_sb[:], ins["leak"][:, :])
    ones64 = mk(sm, [1, F], "ones64")
    nc.vector.memset(ones64[:], 1.0)
    ps_l = mk(psp, [F, 1], "we", bufs=2)
    mmr(ps_l[:], ones64[:], leak_sb[:], True, True)
    leak64 = mk(sm, [F, 1], "leak64")
    nc.scalar.copy(leak64[:], ps_l[:])

    latd = []    # [l][kc] -> (128, 2B) bf16
    lat8dr = []  # [l] -> (128, 16) fp8 [kc0 | kc1] for DoubleRow
    latme = []   # [l][kc] -> (128, 1) bf16 (this core's sample)
    biasW = []   # [l][kc] -> (128, 480) bf16
    for l in range(L):
        lnT, lm, bwl = [], [], []
        t8 = mk(sm, [128, 16], f"lat8dr{l}", f8)
        for kc in range(2):
            tf = mk(sm, [128, 2 * B], f"latnf{l}{kc}")
            nc.sync.dma_start(tf[:],
                              ins["latn"][l, kc * 128:(kc + 1) * 128, :])
            t = mk(sm, [128, 2 * B], f"latd{l}{kc}", bf16)
            nc.vector.tensor_copy(t[:], tf[:])
            nc.vector.tensor_copy(t8[:, 8 * kc:8 * kc + 8], tf[:])
            mf = mk(sm, [128, 1], f"latmf{l}{kc}")
            nc.sync.dma_start(mf[:],
                              ins["latnme"][l, kc * 128:(kc + 1) * 128, :])
            m = mk(sm, [128, 1], f"latme{l}{kc}", bf16)
            nc.vector.tensor_copy(m[:], mf[:])
            bw = mk(sm, [128, 480], f"biasW{l}{kc}", bf16)
            nc.sync.dma_start(bw[:],
                              ins["biasW"][l, kc * 128:(kc + 1) * 128, :])
            lnT.append(t)
            lm.append(m)
            bwl.append(bw)
        latd.append(lnT)
        lat8dr.append(t8)
        latme.append(lm)
        biasW.append(bwl)

    imgWT = mk(sm, [F, 3], "imgWT")
    nc.sync.dma_start(imgWT[:], ins["imgWT"][:, :])
    imgWT_bf = mk(sm, [F, 3], "imgWT_bf", f16)
    nc.vector.tensor_copy(imgWT_bf[:], imgWT[:])
    imgb = mk(sm, [3, 1], "imgb")
    nc.sync.dma_start(imgb[:], ins["imgb"][:, :])

    seedt = mk(sm, [F, 256], "seedt")
    nc.sync.dma_start(seedt[:], ins["seed"][:, :])
    out_bf = mk(big, [F, HWS[-1]], "obf", f16)
    nc.vector.tensor_copy(out_bf[:, 0:256], seedt[:])

    # ---------------- phase A
    conA = [None] * L
    conB = [None] * L
    gatA = [None] * L
    gatB = [None] * L
    wbt_tiles = [None] * L
    cw_tiles = [None] * L
    pending = []     # deferred group closures (layer 2)
    di = {"n": 0}

    def emit_cc(l, part):
        i_ap = (conA if part == 0 else conB)[l]
        o_ap = (gatA if part == 0 else gatB)[l]
        nc.gpsimd.collective_compute(
            "AllToAll", mybir.AluOpType.bypass,
            replica_groups=[list(range(NCORE))],
            ins=[i_ap[:].opt()],
            outs=[o_ap[:].opt()],
        )

    def phase_a_emit(l, defer=False):
        conA[l] = mk(dram, [NCORE, P1_CON], f"conA{l}", bf16)
        conB[l] = mk(dram, [NCORE, P2_CON], f"conB{l}", bf16)
        gatA[l] = mk(dram, [NCORE, P1_CON], f"gatA{l}", bf16)
        gatB[l] = mk(dram, [NCORE, P2_CON], f"gatB{l}", bf16)
        # stream loads (sync queue), in group-consumption order
        t8t = [None, None]
        tbt = [[None, None] for _ in range(4)]
        for ch in range(2):
            t = mk(pha, [128, 8192], "t8", f8, bufs=2)
            nc.sync.dma_start(
                t[:], ins["wws8"][l, :, ch * 8192:(ch + 1) * 8192])
            t8t[ch] = t
        for ch in range(4):
            cw_ = 2048 if ch == 3 else 4096
            for kc in range(2):
                t = mk(pha, [128, 4096], f"tb_{kc}", bf16, bufs=2)
                nc.sync.dma_start(
                    t[:, :cw_], ins["wwsb"][l, kc * 128:(kc + 1) * 128,
                                            ch * 4096:ch * 4096 + cw_])
                tbt[ch][kc] = t
        # wave-bias + cell-wave loads for this layer (sync queue)
        hw = HWS[l]
        if l < 2:
            t = mk(pha, [128, 8 * hw], f"wb{l}", f8)
            nc.sync.dma_start(t[:], ins[f"wb{l}"][:, :])
            wbt_tiles[l] = [t, t]
        else:
            ts = []
            for q in range(4):
                t = mk(pha, [128, 8192], "wb2", f8, bufs=2)
                nc.sync.dma_start(t[:],
                                  ins["wb2"][:, q * 8192:(q + 1) * 8192])
                ts.append(t)
            wbt_tiles[l] = ts
        cwt = mk(pha, [CPE, hw], f"cw{l}", f16)
        nc.sync.dma_start(cwt[:], ins[f"cw{l}"][:, :])
        cw_tiles[l] = cwt

        def make_group(g):
            def emit():
                # psum base partition must be 0/32/64: 3 subs in one bank,
                # the 4th in a second bank
                psA = mk(psp, [128, 512], "pa", bufs=2)
                psB = mk(psp, [128, 512], "pa", bufs=2)
                if g < 4:
                    # fp8 (DoubleRow hits an ISA check at psum base 32/64;
                    # plain fp8 with slot-sliced DR layout instead)
                    for u in range(4):
                        ps, pb = (psA, 32 * u) if u < 3 else (psB, 0)
                        cdr = 4 * g + u
                        tile = t8t[cdr // 8]
                        lo = (cdr % 8) * 1024
                        for i in range(2):
                            nc.tensor.matmul(
                                ps[pb:pb + 2 * B, :],
                                lat8dr[l][:, 8 * i:8 * i + 8],
                                tile[:, lo + 512 * i:lo + 512 * (i + 1)],
                                start=(i == 0), stop=(i == 1),
                                skip_group_check=True)
                else:
                    wg = g - 4
                    rhs = tbt[wg // 2]
                    roff = (wg % 2) * 2048
                    for kc in range(2):
                        for u in range(4):
                            ps, pb = (psA, 32 * u) if u < 3 else (psB, 0)
                            nc.tensor.matmul(
                                ps[pb:pb + 2 * B, :],
                                latd[l][kc][:, :],
                                rhs[kc][:, roff + 512 * u:roff + 512 * (u + 1)],
                                start=(kc == 0), stop=(kc == 1),
                                skip_group_check=True)
                stage = mk(stg, [2 * B, G], "stage", bf16)
                for u in range(4):
                    ps, pb = (psA, 32 * u) if u < 3 else (psB, 0)
                    if di["n"] % 2 == 0:
                        nc.vector.tensor_copy(stage[:, 512 * u:512 * (u + 1)],
                                              ps[pb:pb + 2 * B, :])
                    else:
                        nc.scalar.copy(stage[:, 512 * u:512 * (u + 1)],
                                       ps[pb:pb + 2 * B, :])
                    di["n"] += 1
                c0 = G * g
                if g < 6:
                    nc.gpsimd.dma_start(conA[l][:, c0:c0 + G], stage[:, :])
                elif g == 6:
                    nc.gpsimd.dma_start(conA[l][:, c0:P1_CON],
                                        stage[:, 0:P1_CON - c0])
                    nc.gpsimd.dma_start(conB[l][:, 0:c0 + G - P1_CON],
                                        stage[:, P1_CON - c0:G])
                else:
                    nc.gpsimd.dma_start(
                        conB[l][:, c0 - P1_CON:c0 - P1_CON + G], stage[:, :])
                if g == 6:
                    emit_cc(l, 0)
                elif g == NGROUP - 1:
                    emit_cc(l, 1)
            return emit

        groups = [make_group(g) for g in range(NGROUP)]
        if defer:
            pending.extend(groups)
        else:
            for fn in groups:
                fn()

    def drain(k):
        for _ in range(min(k, len(pending))):
            pending.pop(0)()

    # ---------------- weight loads (scalar queue; dependency-aligned)
    def load_conv(l, name, fin, fout):
        rows = min(fin, 128)
        nkc = fin // rows
        do = fout // NCORE
        part1 = any(name == n for n, _f, _o in P1_CONVS)
        gt = (gatA if part1 else gatB)[l]
        tiles = []
        for kc in range(nkc):
            t = mk(gw, [rows, fout], f"w_{name}{kc}", bf16)
            off = WSB_OFF[(name, kc)]
            gcol = BT_W + off if part1 else off - P1B
            src = gt[0:NCORE, gcol:gcol + rows * do].rearrange(
                "r (i o) -> i r o", o=do)
            dst = t[:].rearrange("i (r o) -> i r o", o=do)
            nc.scalar.dma_start(dst, src)
            tiles.append(t)
        return tiles

    def gen_bias(l, name, fout, tag=None):
        """Generate this core's sample's bias vector on-device:
        bias = biasW[:, cum:cum+fout].T @ lat_new_me  (no A2A involved)."""
        cum = BIAS_CUM[name]
        ps = mk(psp, [fout, 1], "we", bufs=2)
        for kc in range(2):
            mmr(ps[:], biasW[l][kc][:, cum:cum + fout], latme[l][kc][:],
                kc == 0, kc == 1)
        t = mk(gw, [fout, 1], tag or f"b_{name}")
        nc.scalar.copy(t[:], ps[:])
        return t

    def prefetch_bias(l):
        """Bias generation: local matmuls only, no collective dependency."""
        w = {}
        btb = gen_bias(l, "bt", 64, tag="btb")
        bt_b2 = mk(gw, [128, 1], "bt_b2")
        nc.vector.tensor_copy(bt_b2[0:64, :], btb[:])
        nc.vector.tensor_copy(bt_b2[64:128, :], btb[:])
        nc.vector.tensor_scalar_add(bt_b2[0:64, :], bt_b2[0:64, :], PI_2)
        w["bt_b2"] = bt_b2
        trig_b = mk(gw, [128, 1], "trig_b")
        nc.scalar.activation(trig_b[:], bt_b2[:], ACTF.Sin)
        w["trig_b"] = trig_b
        w["c2w_b"] = gen_bias(l, "c2w", 32)
        skip_b = gen_bias(l, "skip", 64)
        w["c1_b"] = gen_bias(l, "c1", 128)
        w["c2_b"] = gen_bias(l, "c2", 128)
        c3_b = gen_bias(l, "c3", 64)
        sbc3 = mk(gw, [F, 1], "sbc3")
        nc.vector.tensor_add(sbc3[:], skip_b[:], c3_b[:])
        w["sbc3"] = sbc3
        return w

    def prefetch_p1(l, w):
        """bt + pass-1 conv loads (need ccA only)."""
        btall = mk(gw, [128, 512], "btall", bf16)
        nc.scalar.dma_start(
            btall[:].rearrange("i (r o) -> i r o", o=64),
            gatA[l][0:NCORE, 0:BT_W].rearrange("r (i o) -> i r o", o=64))
        btd = []
        for j in range(4):
            # fp8 DoubleRow lhsT: [block(2j) cos|sin | block(2j+1) cos|sin]
            t = mk(gw, [128, 256], f"btdr{j}", f8)
            for i in range(2):
                kc = 2 * j + i
                nc.vector.tensor_copy(t[:, 128 * i:128 * i + 64],
                                      btall[:, 64 * kc:64 * kc + 64])
                nc.vector.tensor_copy(t[:, 128 * i + 64:128 * i + 128],
                                      btall[:, 64 * kc:64 * kc + 64])
            btd.append(t)
        w["bt"] = btd
        c2w = load_conv(l, "c2w", 64, 32)[0]
        c2wf = mk(gw, [64, 32], "c2wf")
        nc.vector.tensor_copy(c2wf[:], c2w[:])
        w["c2w"] = c2wf
        w["w2cA"] = load_conv(l, "w2cA", 32, 128)[0]
        w["w2cB"] = load_conv(l, "w2cB", 128, 128)[0]
        g2c = load_conv(l, "g2c", 128, 128)[0]
        wAB = mk(gw, [128, 128], "wAB", f16)
        nc.vector.tensor_scalar(wAB[:], g2c[:], 0.125, None, ALU.mult)
        w["wAB"] = wAB
        w["cw"] = cw_tiles[l]
        return w

    def prefetch_p2(l, w):
        """Residual-block conv loads (need ccB)."""
        w["skip"] = load_conv(l, "skip", 256, 64)
        w["c1"] = load_conv(l, "c1", 256, 128)
        w["c2"] = load_conv(l, "c2", 128, 128)[0]
        w["c3"] = load_conv(l, "c3", 128, 64)[0]
        return w

    # ---------------- surf -> packed cos/sin tile (128 partitions)
    def emit_surf(l, w):
        hw = HWS[l]
        cs = 256 if l == 0 else 512        # spatial chunk width
        nch = hw // cs
        cossin = mk(big, [128, hw], "cossin", f16)
        for c in range(nch):
            if l == 2:
                tile, cl = wbt_tiles[l][c // 2], c % 2
            else:
                tile, cl = wbt_tiles[l][0], c
            ps = mk(psp, [128, 512], "we", bufs=2)
            for j in range(4):
                lo = cl * 8 * cs + j * 2 * cs
                nc.tensor.matmul(
                    ps[:, :cs],
                    w["bt"][j][:].rearrange("p (two m) -> p two m", two=2),
                    tile[:, lo:lo + 2 * cs].rearrange(
                        "p (two n) -> p two n", two=2),
                    start=(j == 0), stop=(j == 3),
                    perf_mode=mybir.MatmulPerfMode.DoubleRow)
            tt = mk(ab, [128, 512], "trigtmp")
            nc.scalar.activation(tt[:, :cs], ps[:, :cs], ACTF.Sin,
                                 bias=w["bt_b2"][:], scale=SIN_SCALE)
            nc.vector.tensor_scalar(cossin[:, c * cs:(c + 1) * cs],
                                    tt[:, :cs], w["trig_b"][:], None,
                                    ALU.subtract)
        return cossin

    # ---------------- DVE rsqrt (quake + 2 newton steps)
    def rsqrt_dve(ve, nm):
        y = mk(smc, [128, 2], f"rsq{nm}")
        yi = y[:].bitcast(i32)
        nc.vector.tensor_scalar(yi, ve[:].bitcast(i32), 1, None,
                                ALU.arith_shift_right)
        nc.vector.tensor_scalar(yi, yi, RSQRT_MAGIC, None, ALU.subtract)
        nc.vector.tensor_scalar(yi, yi, -1, None, ALU.mult)
        t = mk(smc, [128, 2], f"rsqt{nm}")
        for _ in range(2):
            nc.vector.tensor_mul(t[:], y[:], y[:])
            nc.vector.tensor_mul(t[:], t[:], ve[:])
            nc.vector.tensor_scalar(t[:], t[:], -0.5, 1.5, ALU.mult, ALU.add)
            nc.vector.tensor_mul(y[:], y[:], t[:])
        return y

    # ---------------- emit everything
    w = prefetch_bias(0)
    phase_a_emit(0)
    phase_a_emit(1)
    phase_a_emit(2, defer=True)
    w = prefetch_p1(0, w)

    for l in range(L):
        s = SIZES[l]
        hw = HWS[l]
        nch = max(1, hw // 512)
        cn0 = min(512, hw)

        cossin = emit_surf(l, w)

        img_sb = None
        if l == L - 1:
            img_sb = mk(big, [3, hw], "img", f16)

        for call in range(2):
            do_drain = (l, call) >= (0, 1)
            # ---- sobel prep (f16 shifted adds)
            d3 = mk(big, [F, hw], "d3", f16)
            f3p = mk(big, [F, hw], "f3p", f16)
            o3 = mk(big, [F, hw], "o3", f16)
            U = mk(big, [128, hw], "upk", f16)
            bf = out_bf
            d33 = d3[:].rearrange("p (h w) -> p h w", h=s)
            f33 = f3p[:].rearrange("p (h w) -> p h w", h=s)
            bf3 = out_bf[:, 0:hw].rearrange("p (h w) -> p h w", h=s)
            nc.vector.tensor_sub(d3[:, 1:hw - 1], bf[:, 2:hw], bf[:, 0:hw - 2])
            nc.vector.tensor_copy(d33[:, :, 0:1], bf3[:, :, 1:2])
            nc.vector.tensor_scalar(d33[:, :, s - 1:s], bf3[:, :, s - 2:s - 1],
                                    -1.0, None, ALU.mult)
            nc.vector.tensor_add(f3p[:, 1:hw - 1], bf[:, 2:hw],
                                 bf[:, 0:hw - 2])
            nc.vector.tensor_copy(f33[:, :, 0:1], bf3[:, :, 1:2])
            nc.vector.tensor_copy(f33[:, :, s - 1:s], bf3[:, :, s - 2:s - 1])
            nc.vector.tensor_scalar(o3[:, :], bf[:, 0:hw], 2.0, None, ALU.mult)
            nc.vector.tensor_add(o3[:, :], o3[:, :], f3p[:, :])
            # wave coefficients
            so = mk(smc, [F, 1], "so")
            nc.vector.tensor_reduce(so[:], out_bf[:, 0:hw], axis=AX.X,
                                    op=ALU.add)
            ps_c = mk(psp, [CPE, 1], "we", bufs=2)
            mmr(ps_c[:], w["c2w"][:], so[:], True, True)
            coef = mk(smc, [CPE, 1], "coef")
            nc.scalar.activation(coef[:], ps_c[:], ACTF.Identity,
                                 bias=w["c2w_b"][:])
            w2cA_eff = mk(gw, [CPE, 128], "w2cAe", f16)
            nc.vector.tensor_scalar(w2cA_eff[:], w["w2cA"][:], coef[:], None,
                                    ALU.mult)

            def emit_U(a, b):
                ua, ub = max(a, s), min(b, hw - s)
                nc.vector.tensor_add(U[0:F, ua:ub], d3[:, ua - s:ub - s],
                                     d3[:, ua + s:ub + s])
                if a == 0:
                    nc.vector.tensor_copy(U[0:F, 0:s], d3[:, s:2 * s])
                if b == hw:
                    nc.vector.tensor_copy(U[0:F, hw - s:hw],
                                          d3[:, hw - 2 * s:hw - s])
                d2t = mk(big, [F, hw], "d2t", f16)
                nc.vector.tensor_scalar(d2t[:, a:b], d3[:, a:b], 2.0, None,
                                        ALU.mult)
                nc.vector.tensor_add(U[0:F, a:b], U[0:F, a:b], d2t[:, a:b])
                nc.vector.tensor_sub(U[F:128, ua:ub], o3[:, ua + s:ub + s],
                                     o3[:, ua - s:ub - s])
                if a == 0:
                    nc.vector.tensor_copy(U[F:128, 0:s], o3[:, s:2 * s])
                if b == hw:
                    nc.vector.tensor_scalar(U[F:128, hw - s:hw],
                                            o3[:, hw - 2 * s:hw - s], -1.0,
                                            None, ALU.mult)

            # ---- pass 1: ge/we raw matmuls + stats
            we_raw = mk(big, [128, hw], "weraw", f16)
            ge_raw = mk(big, [128, hw], "geraw", f16)
            we_st = mk(smc, [128, 6 * nch], "west")
            ge_st = mk(smc, [128, 6 * nch], "gest")
            for n in range(nch):
                c0 = n * cn0
                cn = cn0
                if n % 4 == 0:
                    emit_U(c0, min(c0 + 4 * cn0, hw))
                ps_ge = mk(psp, [128, 512], "g1", bufs=2)
                mmr(ps_ge[:, :cn], w["wAB"][:], U[:, c0:c0 + cn], True, True)
                ps_we = mk(psp, [128, 512], "we", bufs=2)
                mmr(ps_we[:, :cn], w2cA_eff[:], w["cw"][:, c0:c0 + cn],
                    True, False)
                mmr(ps_we[:, :cn], w["w2cB"][:], cossin[:, c0:c0 + cn],
                    False, True)
                nc.vector.bn_stats(ge_st[:, n * 6:(n + 1) * 6], ps_ge[:, :cn])
                nc.scalar.copy(ge_raw[:, c0:c0 + cn], ps_ge[:, :cn])
                nc.vector.bn_stats(we_st[:, n * 6:(n + 1) * 6], ps_we[:, :cn])
                nc.scalar.copy(we_raw[:, c0:c0 + cn], ps_we[:, :cn])
                if do_drain:
                    drain(1)

            if call == 0:
                w = prefetch_p2(l, w)

            # ---- fold instance norm into the skip/c1 weights
            mv = mk(smc, [128, 4], "mv")
            nc.vector.bn_aggr(mv[:, 0:2], ge_st[:])
            nc.vector.bn_aggr(mv[:, 2:4], we_st[:])
            ve = mk(smc, [128, 2], "ve")
            nc.vector.tensor_scalar(ve[:, 0:1], mv[:, 1:2], 1e-5, None,
                                    ALU.add)
            nc.vector.tensor_scalar(ve[:, 1:2], mv[:, 3:4], 1e-5, None,
                                    ALU.add)
            rs = rsqrt_dve(ve, "a")
            m_bf = mk(smc, [128, 2], "m_bf", f16)
            nc.vector.tensor_copy(m_bf[:, 0:1], mv[:, 0:1])
            nc.vector.tensor_copy(m_bf[:, 1:2], mv[:, 2:3])
            sk0s = mk(gw, [128, F], "sk0s", f16)
            nc.vector.tensor_scalar(sk0s[:], w["skip"][0][:], rs[:, 0:1],
                                    None, ALU.mult)
            sk1s = mk(gw, [128, F], "sk1s", f16)
            nc.vector.tensor_scalar(sk1s[:], w["skip"][1][:], rs[:, 1:2],
                                    None, ALU.mult)
            c10s = mk(gw, [128, 128], "c10s", f16)
            nc.vector.tensor_scalar(c10s[:], w["c1"][0][:], rs[:, 0:1],
                                    None, ALU.mult)
            c11s = mk(gw, [128, 128], "c11s", f16)
            nc.vector.tensor_scalar(c11s[:], w["c1"][1][:], rs[:, 1:2],
                                    None, ALU.mult)
            ps_f1 = mk(psp, [F, 1], "sc", bufs=2)
            mmr(ps_f1[:], sk0s[:], m_bf[:, 0:1], True, False)
            mmr(ps_f1[:], sk1s[:], m_bf[:, 1:2], False, True)
            bias_sc = mk(smc, [F, 1], "bias_sc")
            nc.vector.tensor_sub(bias_sc[:], w["sbc3"][:], ps_f1[:])
            nc.vector.tensor_mul(bias_sc[:], bias_sc[:], leak64[:])
            ps_f2 = mk(psp, [128, 1], "sc", bufs=2)
            mmr(ps_f2[:], c10s[:], m_bf[:, 0:1], True, False)
            mmr(ps_f2[:], c11s[:], m_bf[:, 1:2], False, True)
            c1_be = mk(smc, [128, 1], "c1_be")
            nc.vector.tensor_sub(c1_be[:], w["c1_b"][:], ps_f2[:])
            if do_drain:
                drain(1)

            # ---- pass 2: skip/c1 -> prelu -> c2 -> prelu -> c3, residual
            ps_s = [None] * nch
            ps_1 = [None] * nch

            def emit_front(n):
                c0 = n * cn0
                cn = cn0
                ps_s[n] = mk(psp, [F, 512], "sc", bufs=2)
                mmr(ps_s[n][:, :cn], sk0s[:], ge_raw[:, c0:c0 + cn],
                    True, False)
                mmr(ps_s[n][:, :cn], sk1s[:], we_raw[:, c0:c0 + cn],
                    False, False)
                ps_1[n] = mk(psp, [128, 512], "g1", bufs=2)
                mmr(ps_1[n][:, :cn], c10s[:], ge_raw[:, c0:c0 + cn],
                    True, False)
                mmr(ps_1[n][:, :cn], c11s[:], we_raw[:, c0:c0 + cn],
                    False, True)

            def lrelu_act(dst, src, bias):
                if SIM_COMPAT:
                    nc.scalar.activation(dst, src, ACTF.Identity, bias=bias)
                    nc.vector.scalar_tensor_tensor(dst, dst, 0.2, dst,
                                                   ALU.mult, ALU.max)
                else:
                    nc.scalar.activation(dst, src, ACTF.Prelu, bias=bias,
                                         alpha=0.2)

            def emit_back(n):
                c0 = n * cn0
                cn = cn0
                a1 = mk(ab, [128, 512], "a1", f16)
                lrelu_act(a1[:, :cn], ps_1[n][:, :cn], c1_be[:])
                ps_2 = mk(psp, [128, 512], "we", bufs=2)
                mmr(ps_2[:, :cn], w["c2"][:], a1[:, :cn], True, True)
                a2 = mk(ab, [128, 512], "a2", f16)
                lrelu_act(a2[:, :cn], ps_2[:, :cn], w["c2_b"][:])
                mmr(ps_s[n][:, :cn], w["c3"][:], a2[:, :cn], False, True)
                tn = mk(ab, [F, 512], "tn", f16)
                nc.scalar.activation(tn[:, :cn], ps_s[n][:, :cn],
                                     ACTF.Identity, bias=bias_sc[:],
                                     scale=leak64[:])
                if l == L - 1:
                    nc.gpsimd.tensor_add(out_bf[:, c0:c0 + cn],
                                         out_bf[:, c0:c0 + cn], tn[:, :cn])
                else:
                    nc.vector.tensor_add(out_bf[:, c0:c0 + cn],
                                         out_bf[:, c0:c0 + cn], tn[:, :cn])
                if l == L - 1 and call == 1:
                    ps_i = mk(psp, [3, 512], "g1", bufs=2)
                    mmr(ps_i[:, :cn], imgWT_bf[:], out_bf[:, c0:c0 + cn],
                        True, True)
                    nc.scalar.activation(img_sb[:, c0:c0 + cn], ps_i[:, :cn],
                                         ACTF.Identity, bias=imgb[:])
                    nc.vector.tensor_scalar(img_sb[:, c0:c0 + cn],
                                            img_sb[:, c0:c0 + cn], -1.0, 1.0,
                                            ALU.max, ALU.min)

            for n in range(nch + 1):
                if n < nch:
                    emit_front(n)
                if n >= 1:
                    emit_back(n - 1)
                    if do_drain:
                        drain(1)
            if do_drain:
                drain(2)

        # -------- layer transition: bilinear 2x upsample + gauss (f16)
        if l < L - 1:
            s2 = 2 * s
            o3v = out_bf[:, 0:hw].rearrange("p (h w) -> p h w", h=s)
            po = mk(big, [F, hw], "d3", f16)
            nc.gpsimd.tensor_scalar(po[:, :], out_bf[:, 0:hw], 1.0 / 3.0, None,
                                    ALU.mult)
            po3 = po[:].rearrange("p (h w) -> p h w", h=s)
            upv = mk(big, [F, s2 * s], "f3p", f16)
            v3 = upv[:].rearrange("p (h w) -> p h w", h=s2)
            nc.vector.tensor_add(v3[:, 2:s2:2, :], o3v[:, 1:s, :],
                                 po3[:, 0:s - 1, :])
            nc.vector.tensor_add(v3[:, 1:s2 - 1:2, :], o3v[:, 0:s - 1, :],
                                 po3[:, 1:s, :])
            nc.vector.tensor_scalar(v3[:, 0:1, :], o3v[:, 0:1, :], 4.0 / 3.0,
                                    None, ALU.mult)
            nc.vector.tensor_scalar(v3[:, s2 - 1:s2, :], o3v[:, s - 1:s, :],
                                    4.0 / 3.0, None, ALU.mult)
            ph = mk(big, [F, s2 * s], "o3", f16)
            nc.gpsimd.tensor_scalar(ph[:, :s2 * s], upv[:, :s2 * s],
                                    1.0 / 3.0, None, ALU.mult)
            ph3 = ph[:].rearrange("p (h w) -> p h w", h=s2)
            uph = mk(big, [F, s2 * s2], "weraw", f16)
            h3 = uph[:].rearrange("p (h w) -> p h w", h=s2)
            nc.vector.tensor_add(h3[:, :, 2:s2:2], v3[:, :, 1:s],
                                 ph3[:, :, 0:s - 1])
            nc.vector.tensor_add(h3[:, :, 1:s2 - 1:2], v3[:, :, 0:s - 1],
                                 ph3[:, :, 1:s])
            nc.vector.tensor_scalar(h3[:, :, 0:1], v3[:, :, 0:1], 4.0 / 3.0,
                                    None, ALU.mult)
            nc.vector.tensor_scalar(h3[:, :, s2 - 1:s2], v3[:, :, s - 1:s],
                                    4.0 / 3.0, None, ALU.mult)
            hw2 = s2 * s2
            gu = mk(big, [F, hw2], "geraw", f16)
            nc.vector.tensor_add(gu[:, s2:hw2 - s2], uph[:, 0:hw2 - 2 * s2],
                                 uph[:, 2 * s2:hw2])
            nc.vector.tensor_copy(gu[:, 0:s2], uph[:, s2:2 * s2])
            nc.vector.tensor_copy(gu[:, hw2 - s2:hw2],
                                  uph[:, hw2 - 2 * s2:hw2 - s2])
            hg = mk(big, [F, hw2], "d2t", f16)
            nc.gpsimd.tensor_scalar(hg[:, :], uph[:, :], GRBA, None, ALU.mult)
            nc.vector.tensor_add(gu[:, :], gu[:, :], hg[:, :])
            gu3 = gu[:].rearrange("p (h w) -> p h w", h=s2)
            gu2 = mk(big, [F, hw2], "upk", f16)
            g23 = gu2[:].rearrange("p (h w) -> p h w", h=s2)
            nc.vector.tensor_add(g23[:, :, 1:s2 - 1], gu3[:, :, 0:s2 - 2],
                                 gu3[:, :, 2:s2])
            nc.vector.tensor_copy(g23[:, :, 0:1], gu3[:, :, 1:2])
            nc.vector.tensor_copy(g23[:, :, s2 - 1:s2], gu3[:, :, s2 - 2:s2 - 1])
            hg2 = mk(big, [F, hw2], "cossin", f16)
            nc.gpsimd.tensor_scalar(hg2[:, :], gu[:, :], GRBA, None, ALU.mult)
            nc.vector.tensor_add(gu2[:, :], gu2[:, :], hg2[:, :])
            newout = mk(big, [F, HWS[-1]], "obf", f16)
            nc.vector.tensor_scalar(newout[:, :hw2], gu2[:, :], UPGAUSS_C,
                                    None, ALU.mult)
            out_bf = newout
            # all pending phase-A groups must be emitted before prefetch(2):
            # its scalar-queue DMAs wait on ccA2/ccB2, which need those
            # groups' (scalar-queue) stage copies — emitting them after
            # would deadlock the scalar queue.
            drain(3 if l == 0 else 99)
            w = prefetch_bias(l + 1)
            w = prefetch_p1(l + 1, w)
        else:
            nc.gpsimd.dma_start(out_img[:, :], img_sb[:])

    for p in reversed(ctxs):
        p.__exit__(None, None, None)


_MODULE_CACHE = {}


def build_module():
    if "nc" in _MODULE_CACHE:
        return _MODULE_CACHE["nc"]
    from concourse import bacc, mybir
    import concourse.tile as tile

    nc = bacc.Bacc("TRN2", target_bir_lowering=False, debug=False,
                   num_devices=NCORE)
    f32 = mybir.dt.float32
    dts = {"f32": mybir.dt.float32, "bf16": mybir.dt.bfloat16,
           "f16": mybir.dt.float16, "f8": mybir.dt.float8e4}
    ins = {}
    for name, shape, dt in INPUT_SHAPES:
        ins[name] = nc.dram_tensor(name, list(shape), dts[dt],
                                   kind="ExternalInput").ap()
    out_img = nc.dram_tensor("img", [3, HWS[-1]], mybir.dt.float16,
                             kind="ExternalOutput").ap()
    with tile.TileContext(nc) as tc:
        build_kernel(tc, ins, out_img)
    nc.compile()
    _MODULE_CACHE["nc"] = nc
    return nc


# ---------------------------------------------------------------- host prep
def host_prep(inputs):
    """Full (unsharded) numpy inputs -> per-core in_maps."""
    import ml_dtypes

    bf16 = ml_dtypes.bfloat16
    fp8 = ml_dtypes.float8_e4m3

    lat = np.asarray(inputs["lat"], np.float32)
    leak = float(np.clip(np.asarray(inputs["leak_factor"]), 0.001, 1000.0))
    seed = np.asarray(inputs["seed"], np.float32)[0].reshape(F, 256)
    frac_W = np.asarray(inputs["frac_W"], np.float32)
    frac_b = np.asarray(inputs["frac_b"], np.float32)

    def T(W, fin, fout):
        W = np.asarray(W, np.float32)
        return W.reshape(L, LAT, fout, fin).transpose(0, 1, 3, 2)

    bt_T = T(inputs["bt_W"], 1024, F)
    c2w_T = T(inputs["c2w_W"], F, CPE).copy()
    for l in range(L):
        c2w_T[l] *= 1.0 / HWS[l]
    w2c_T = T(inputs["w2c_W"], CPE + 2 * F, 2 * F)
    convT = {
        "c2w": c2w_T,
        "w2cA": w2c_T[:, :, :CPE, :],
        "w2cB": np.ascontiguousarray(w2c_T[:, :, CPE:, :]) * np.float32(E1),
        "g2c": T(inputs["g2c_W"], 2 * F, 2 * F),
        "skip": T(inputs["cc_skip_W"], 4 * F, F),
        "c1": T(inputs["cc_c1_W"], 4 * F, 2 * F),
        "c2": T(inputs["cc_c2_W"], 2 * F, 2 * F),
        "c3": T(inputs["cc_c3_W"], 2 * F, F),
    }
    # lat_new per layer/sample (tiny host matmul — input prep), doubled cols
    lat_new = np.stack([lat @ frac_W[l] + frac_b[l] for l in range(L)])
    latn = np.concatenate([lat_new.transpose(0, 2, 1)] * 2, axis=2)
    latn = np.ascontiguousarray(latn)          # (L, LAT, 2B)
    bias_blk = np.concatenate(
        [np.asarray(inputs[k], np.float32) for _n, k, _fo in BIAS_KEYS],
        axis=2)                                # (L, LAT, 480)

    common = {
        "latn": latn,
        "biasW": bias_blk.astype(bf16),
        "leak": np.full((1, 1), leak, np.float32),
        "seed": np.ascontiguousarray(seed),
        "imgWT": np.ascontiguousarray(np.asarray(inputs["img_W"],
                                                 np.float32).T),
        "imgb": np.asarray(inputs["img_b"], np.float32).reshape(3, 1),
    }
    for l in range(L):
        hw = HWS[l]
        cs = 256 if l == 0 else 512
        wbf = np.asarray(inputs[f"wave_bias{l}"], np.float32)[0].reshape(
            F * 16, hw)
        # (128, nch, jpair, slot, cs): DoubleRow rhs layout
        packed = wbf.reshape(4, 2, 128, hw // cs, cs).transpose(
            2, 3, 0, 1, 4).reshape(128, 8 * hw)
        common[f"wb{l}"] = np.ascontiguousarray(packed * S_WB).astype(fp8)
        common[f"cw{l}"] = pos_enc_np(SIZES[l]).astype(np.float16)

    in_maps = []
    for r in range(NCORE):
        m = dict(common)
        m["latnme"] = np.ascontiguousarray(
            lat_new[:, r % B, :][:, :, None])   # (L, LAT, 1)
        w8 = bt_T[:, :, r * 128:(r + 1) * 128, :].reshape(L, LAT, BT_W)
        # DoubleRow rhs layout: (128, chunk, slot, 512), slot = LAT half
        w8 = w8.reshape(L, 2, 128, BT_W // 512, 512).transpose(
            0, 2, 3, 1, 4).reshape(L, 128, 2 * BT_W)
        m["wws8"] = np.ascontiguousarray(w8 * S_BT).astype(fp8)
        wsb = np.zeros((L, LAT, WSB_W), np.float32)
        for n, fi, fo in P1_CONVS + P2_CONVS:
            rows = min(fi, 128)
            do = fo // NCORE
            for kc in range(fi // rows):
                off = WSB_OFF[(n, kc)]
                wsb[:, :, off:off + rows * do] = convT[n][
                    :, :, kc * rows:(kc + 1) * rows,
                    r * do:(r + 1) * do].reshape(L, LAT, rows * do)
        m["wwsb"] = wsb.astype(bf16)
        in_maps.append(m)
    return in_maps


def kernel(**inputs):
    from concourse.bass_utils import run_bass_kernel_spmd

    nc = build_module()
    in_maps = host_prep(inputs)
    res = run_bass_kernel_spmd(nc, in_maps, list(range(NCORE)))
    imgs = [res.results[b]["img"].reshape(3, SIZES[-1], SIZES[-1])
            for b in range(B)]
    return np.stack(imgs).astype(np.float32)


if __name__ == "__main__":
    nc = build_module()
    print("module built ok; instructions:",
          sum(len(bb.instructions) for bb in nc.main_func.blocks))


# revision 30
# speedup vs baseline: 1.6285x; 1.6285x over previous
"""Trainium2 Bass kernel for nn_Decoder_72911364816952 (v2).

Strategy (8 NeuronCores, memory-bound regime):
  Phase A (all cores): each core reads a distinct 1/8 column-shard of the
    dynamic-weight generator matrices and computes the generated per-sample
    conv weights for all samples. The bt (surf) generator and the wave
    biases are stored fp8 (numerically validated); the rest bf16.
  AllToAll (2 per layer, bf16): routes every sample's generated weights to
    the core that runs that sample. Part 1 carries bt+biases+pass1 convs,
    part 2 the residual-block convs.
  Emission order is the key scheduling device (engine queues are in-order):
    phase A of layers 0 AND 1 is emitted before any phase-B work, so the
    first A2A's global-barrier skew is absorbed by layer-1 generation work;
    layer 2's phase A is drained in small pieces inside phase B of layers
    0/1. Weight loads ride the scalar queue (dependency-aligned with the
    activations that need them), contrib stores + collectives on gpsimd,
    input streaming on sync.
  Phase B (each core, one sample): spatial decoder entirely on-chip with a
    single f16 activation accumulator (validated): fused Sin surf path,
    Sobel as f16 shifted adds + folded channel matmuls, instance norm folded
    into the next conv's weights, residual block in bf16 with Prelu lrelu,
    bilinear-2x + gauss as f16 vector ops.
  Output: each core writes its sample's (3,64,64) image; host stacks 0-3.
"""
import math
import os
import sys

sys.path.insert(0, "/opt/trn_rl_repo")

import numpy as np

# ---------------------------------------------------------------- constants
L = 3
LAT = 256
F = 64
CPE = 32
B = 4
NCORE = 8
SIZES = [16, 32, 64]
HWS = [s * s for s in SIZES]
K_FREQ = 8

S_BT = 512.0           # host scale on bt_W (fp8)
S_WB = 256.0           # host scale on wave biases (fp8)
SIN_SCALE = 1.0 / (S_BT * S_WB)

# per-rank wwsb layout: every rank holds a 1/8 fout-slice of every conv
P1_CONVS = [("c2w", 64, 32), ("w2cA", 32, 128), ("w2cB", 128, 128),
            ("g2c", 128, 128)]
P2_CONVS = [("skip", 256, 64), ("c1", 256, 128), ("c2", 128, 128),
            ("c3", 128, 64)]
BIAS_KEYS = [("bt", "bt_b", 64), ("c2w", "c2w_b", 32),
             ("skip", "cc_skip_b", 64), ("c1", "cc_c1_b", 128),
             ("c2", "cc_c2_b", 128), ("c3", "cc_c3_b", 64)]
BIAS_CUM = {}
_b = 0
for _n, _k, _fo in BIAS_KEYS:
    BIAS_CUM[_n] = _b
    _b += _fo

WSB_OFF = {}
_o = 0
for _n, _fi, _fo in P1_CONVS + P2_CONVS:
    _rows = min(_fi, 128)
    for _kc in range(_fi // _rows):
        WSB_OFF[(_n, _kc)] = _o
        _o += _rows * (_fo // NCORE)
P1B = WSB_OFF[("skip", 0)]         # 4924: wwsb part-1 width
WSB_USED = _o                      # 14140
WSB_W = 14336                      # padded to 2048-multiple
BT_W = 8192
CON_W = BT_W + WSB_W               # 22528
P1_CON = BT_W + P1B                # 13116 (contrib part-1 width)
P2_CON = CON_W - P1_CON
G = 2048                           # phase-A matmul group width
NGROUP = CON_W // G                # 11
E1 = math.e
_ga_raw = math.exp(-0.5)
GA = _ga_raw / (1.0 + 2.0 * _ga_raw)
GRBA = 1.0 / _ga_raw
UPGAUSS_C = 0.5625 * GA * GA
PI_2 = math.pi / 2.0
RSQRT_MAGIC = 0x5F3759DF
SIM_COMPAT = bool(os.environ.get("BASS_SIM_COMPAT"))


def pos_enc_np(size):
    p = np.arange(size, dtype=np.float32)
    feats = []
    for k in range(K_FREQ):
        ang = p * (2.0 ** k) * (2.0 * np.pi / size)
        s, c = np.sin(ang).astype(np.float32), np.cos(ang).astype(np.float32)
        feats += [np.broadcast_to(s[:, None], (size, size)),
                  np.broadcast_to(c[:, None], (size, size)),
                  np.broadcast_to(s[None, :], (size, size)),
                  np.broadcast_to(c[None, :], (size, size))]
    return np.ascontiguousarray(np.stack(feats).reshape(4 * K_FREQ, size * size))


INPUT_SHAPES = [
    ("wws8", (L, 128, 2 * BT_W), "f8"),
    ("wwsb", (L, LAT, WSB_W), "bf16"),
    ("latn", (L, LAT, 2 * B), "f32"),
    ("latnme", (L, LAT, 1), "f32"),
    ("biasW", (L, LAT, 480), "bf16"),
    ("leak", (1, 1), "f32"),
    ("seed", (F, 256), "f32"),
    ("wb0", (128, 8 * HWS[0]), "f8"),
    ("wb1", (128, 8 * HWS[1]), "f8"),
    ("wb2", (128, 8 * HWS[2]), "f8"),
    ("cw0", (CPE, HWS[0]), "f16"),
    ("cw1", (CPE, HWS[1]), "f16"),
    ("cw2", (CPE, HWS[2]), "f16"),
    ("imgWT", (F, 3), "f32"),
    ("imgb", (3, 1), "f32"),
]


# ---------------------------------------------------------------- device code
def build_kernel(tc, ins, out_img):
    import concourse.bass as bass
    from concourse import mybir

    nc = tc.nc
    f32 = mybir.dt.float32
    i32 = mybir.dt.int32
    bf16 = mybir.dt.bfloat16
    f16 = mybir.dt.float16
    f8 = mybir.dt.float8e4
    ALU = mybir.AluOpType
    ACTF = mybir.ActivationFunctionType
    AX = mybir.AxisListType
    ctxs = []

    def pool(name, bufs, space="SBUF"):
        p = tc.tile_pool(name=name, bufs=bufs, space=space)
        ctxs.append(p)
        return p.__enter__()

    def mk(pool_, shape, tag, dt=None, bufs=None, **kw):
        return pool_.tile(shape, dt or f32, name=tag, tag=tag, bufs=bufs, **kw)

    dram = pool("dram", 1, "DRAM")
    psp = pool("psp", 1, "PSUM")
    big = pool("big", 1)
    gw = pool("gw", 1)
    ab = pool("ab", 3)
    sm = pool("sm", 1)
    smc = pool("smc", 2)
    pha = pool("pha", 1)
    stg = pool("stg", 2)

    def mmr(ps, lhsT, rhs, start, stop):
        nc.tensor.matmul(ps, lhsT, rhs, start=start, stop=stop)

    # ---------------- setup: lat_new loads (host-computed), leak
    leak_sb = mk(sm, [1, 1], "leak")
    nc.sync.dma_start(leak_sb[:], ins["leak"][:, :])
    ones64 = mk(sm, [1, F], "ones64")
    nc.vector.memset(ones64[:], 1.0)
    ps_l = mk(psp, [F, 1], "we", bufs=2)
    mmr(ps_l[:], ones64[:], leak_sb[:], True, True)
    leak64 = mk(sm, [F, 1], "leak64")
    nc.scalar.copy(leak64[:], ps_l[:])

    latd = []    # [l][kc] -> (128, 2B) bf16
    lat8dr = []  # [l] -> (128, 16) fp8 [kc0 | kc1] for DoubleRow
    latme = []   # [l][kc] -> (128, 1) bf16 (this core's sample)
    biasW = []   # [l][kc] -> (128, 480) bf16
    for l in range(L):
        lnT, lm, bwl = [], [], []
        t8 = mk(sm, [128, 16], f"lat8dr{l}", f8)
        for kc in range(2):
            tf = mk(sm, [128, 2 * B], f"latnf{l}{kc}")
            nc.sync.dma_start(tf[:],
                              ins["latn"][l, kc * 128:(kc + 1) * 128, :])
            t = mk(sm, [128, 2 * B], f"latd{l}{kc}", bf16)
            nc.vector.tensor_copy(t[:], tf[:])
            nc.vector.tensor_copy(t8[:, 8 * kc:8 * kc + 8], tf[:])
            mf = mk(sm, [128, 1], f"latmf{l}{kc}")
            nc.sync.dma_start(mf[:],
                              ins["latnme"][l, kc * 128:(kc + 1) * 128, :])
            m = mk(sm, [128, 1], f"latme{l}{kc}", bf16)
            nc.vector.tensor_copy(m[:], mf[:])
            bw = mk(sm, [128, 480], f"biasW{l}{kc}", bf16)
            nc.sync.dma_start(bw[:],
                              ins["biasW"][l, kc * 128:(kc + 1) * 128, :])
            lnT.append(t)
            lm.append(m)
            bwl.append(bw)
        latd.append(lnT)
        lat8dr.append(t8)
        latme.append(lm)
        biasW.append(bwl)

    imgWT = mk(sm, [F, 3], "imgWT")
    nc.sync.dma_start(imgWT[:], ins["imgWT"][:, :])
    imgWT_bf = mk(sm, [F, 3], "imgWT_bf", f16)
    nc.vector.tensor_copy(imgWT_bf[:], imgWT[:])
    imgb = mk(sm, [3, 1], "imgb")
    nc.sync.dma_start(imgb[:], ins["imgb"][:, :])

    seedt = mk(sm, [F, 256], "seedt")
    nc.sync.dma_start(seedt[:], ins["seed"][:, :])
    out_bf = mk(big, [F, HWS[-1]], "obf", f16)
    nc.vector.tensor_copy(out_bf[:, 0:256], seedt[:])

    # ---------------- phase A
    conA = [None] * L
    conB = [None] * L
    gatA = [None] * L
    gatB = [None] * L
    wbt_tiles = [None] * L
    cw_tiles = [None] * L
    pending = []     # deferred group closures (layer 2)
    di = {"n": 0}

    def emit_cc(l, part):
        i_ap = (conA if part == 0 else conB)[l]
        o_ap = (gatA if part == 0 else gatB)[l]
        nc.gpsimd.collective_compute(
            "AllToAll", mybir.AluOpType.bypass,
            replica_groups=[list(range(NCORE))],
            ins=[i_ap[:].opt()],
            outs=[o_ap[:].opt()],
        )

    def phase_a_emit(l, defer=False):
        conA[l] = mk(dram, [NCORE, P1_CON], f"conA{l}", bf16)
        conB[l] = mk(dram, [NCORE, P2_CON], f"conB{l}", bf16)
        gatA[l] = mk(dram, [NCORE, P1_CON], f"gatA{l}", bf16)
        gatB[l] = mk(dram, [NCORE, P2_CON], f"gatB{l}", bf16)
        # stream loads (sync queue), in group-consumption order
        t8t = [None, None]
        tbt = [[None, None] for _ in range(4)]
        for ch in range(2):
            t = mk(pha, [128, 8192], "t8", f8, bufs=2)
            nc.sync.dma_start(
                t[:], ins["wws8"][l, :, ch * 8192:(ch + 1) * 8192])
            t8t[ch] = t
        for ch in range(4):
            cw_ = 2048 if ch == 3 else 4096
            for kc in range(2):
                t = mk(pha, [128, 4096], f"tb_{kc}", bf16, bufs=2)
                nc.sync.dma_start(
                    t[:, :cw_], ins["wwsb"][l, kc * 128:(kc + 1) * 128,
                                            ch * 4096:ch * 4096 + cw_])
                tbt[ch][kc] = t
        # wave-bias + cell-wave loads for this layer (sync queue)
        hw = HWS[l]
        if l < 2:
            t = mk(pha, [128, 8 * hw], f"wb{l}", f8)
            nc.sync.dma_start(t[:], ins[f"wb{l}"][:, :])
            wbt_tiles[l] = [t, t]
        else:
            ts = []
            for q in range(4):
                t = mk(pha, [128, 8192], "wb2", f8, bufs=2)
                nc.sync.dma_start(t[:],
                                  ins["wb2"][:, q * 8192:(q + 1) * 8192])
                ts.append(t)
            wbt_tiles[l] = ts
        cwt = mk(pha, [CPE, hw], f"cw{l}", f16)
        nc.sync.dma_start(cwt[:], ins[f"cw{l}"][:, :])
        cw_tiles[l] = cwt

        def make_group(g):
            def emit():
                # psum base partition must be 0/32/64: 3 subs in one bank,
                # the 4th in a second bank
                psA = mk(psp, [128, 512], "pa", bufs=2)
                psB = mk(psp, [128, 512], "pa", bufs=2)
                if g < 4:
                    # fp8 (DoubleRow hits an ISA check at psum base 32/64;
                    # plain fp8 with slot-sliced DR layout instead)
                    for u in range(4):
                        ps, pb = (psA, 32 * u) if u < 3 else (psB, 0)
                        cdr = 4 * g + u
                        tile = t8t[cdr // 8]
                        lo = (cdr % 8) * 1024
                        for i in range(2):
                            nc.tensor.matmul(
                                ps[pb:pb + 2 * B, :],
                                lat8dr[l][:, 8 * i:8 * i + 8],
                                tile[:, lo + 512 * i:lo + 512 * (i + 1)],
                                start=(i == 0), stop=(i == 1),
                                skip_group_check=True)
                else:
                    wg = g - 4
                    rhs = tbt[wg // 2]
                    roff = (wg % 2) * 2048
                    for kc in range(2):
                        for u in range(4):
                            ps, pb = (psA, 32 * u) if u < 3 else (psB, 0)
                            nc.tensor.matmul(
                                ps[pb:pb + 2 * B, :],
                                latd[l][kc][:, :],
                                rhs[kc][:, roff + 512 * u:roff + 512 * (u + 1)],
                                start=(kc == 0), stop=(kc == 1),
                                skip_group_check=True)
                stage = mk(stg, [2 * B, G], "stage", bf16)
                for u in range(4):
                    ps, pb = (psA, 32 * u) if u < 3 else (psB, 0)
                    if di["n"] % 2 == 0:
                        nc.vector.tensor_copy(stage[:, 512 * u:512 * (u + 1)],
                                              ps[pb:pb + 2 * B, :])
                    else:
                        nc.scalar.copy(stage[:, 512 * u:512 * (u + 1)],
                                       ps[pb:pb + 2 * B, :])
                    di["n"] += 1
                c0 = G * g
                if g < 6:
                    nc.gpsimd.dma_start(conA[l][:, c0:c0 + G], stage[:, :])
                elif g == 6:
                    nc.gpsimd.dma_start(conA[l][:, c0:P1_CON],
                                        stage[:, 0:P1_CON - c0])
                    nc.gpsimd.dma_start(conB[l][:, 0:c0 + G - P1_CON],
                                        stage[:, P1_CON - c0:G])
                else:
                    nc.gpsimd.dma_start(
                        conB[l][:, c0 - P1_CON:c0 - P1_CON + G], stage[:, :])
                if g == 6:
                    emit_cc(l, 0)
                elif g == NGROUP - 1:
                    emit_cc(l, 1)
            return emit

        groups = [make_group(g) for g in range(NGROUP)]
        if defer:
            pending.extend(groups)
        else:
            for fn in groups:
                fn()

    def drain(k):
        for _ in range(min(k, len(pending))):
            pending.pop(0)()

    # ---------------- weight loads (scalar queue; dependency-aligned)
    def load_conv(l, name, fin, fout):
        rows = min(fin, 128)
        nkc = fin // rows
        do = fout // NCORE
        part1 = any(name == n for n, _f, _o in P1_CONVS)
        gt = (gatA if part1 else gatB)[l]
        tiles = []
        for kc in range(nkc):
            t = mk(gw, [rows, fout], f"w_{name}{kc}", bf16)
            off = WSB_OFF[(name, kc)]
            gcol = BT_W + off if part1 else off - P1B
            src = gt[0:NCORE, gcol:gcol + rows * do].rearrange(
                "r (i o) -> i r o", o=do)
            dst = t[:].rearrange("i (r o) -> i r o", o=do)
            nc.scalar.dma_start(dst, src)
            tiles.append(t)
        return tiles

    def gen_bias(l, name, fout, tag=None):
        """Generate this core's sample's bias vector on-device:
        bias = biasW[:, cum:cum+fout].T @ lat_new_me  (no A2A involved)."""
        cum = BIAS_CUM[name]
        ps = mk(psp, [fout, 1], "we", bufs=2)
        for kc in range(2):
            mmr(ps[:], biasW[l][kc][:, cum:cum + fout], latme[l][kc][:],
                kc == 0, kc == 1)
        t = mk(gw, [fout, 1], tag or f"b_{name}")
        nc.scalar.copy(t[:], ps[:])
        return t

    def prefetch_bias(l):
        """Bias generation: local matmuls only, no collective dependency."""
        w = {}
        btb = gen_bias(l, "bt", 64, tag="btb")
        bt_b2 = mk(gw, [128, 1], "bt_b2")
        nc.vector.tensor_copy(bt_b2[0:64, :], btb[:])
        nc.vector.tensor_copy(bt_b2[64:128, :], btb[:])
        nc.vector.tensor_scalar_add(bt_b2[0:64, :], bt_b2[0:64, :], PI_2)
        w["bt_b2"] = bt_b2
        trig_b = mk(gw, [128, 1], "trig_b")
        nc.scalar.activation(trig_b[:], bt_b2[:], ACTF.Sin)
        w["trig_b"] = trig_b
        w["c2w_b"] = gen_bias(l, "c2w", 32)
        skip_b = gen_bias(l, "skip", 64)
        w["c1_b"] = gen_bias(l, "c1", 128)
        w["c2_b"] = gen_bias(l, "c2", 128)
        c3_b = gen_bias(l, "c3", 64)
        sbc3 = mk(gw, [F, 1], "sbc3")
        nc.vector.tensor_add(sbc3[:], skip_b[:], c3_b[:])
        w["sbc3"] = sbc3
        return w

    def prefetch_p1(l, w):
        """bt + pass-1 conv loads (need ccA only)."""
        btall = mk(gw, [128, 512], "btall", bf16)
        nc.scalar.dma_start(
            btall[:].rearrange("i (r o) -> i r o", o=64),
            gatA[l][0:NCORE, 0:BT_W].rearrange("r (i o) -> i r o", o=64))
        btd = []
        for kc in range(8):
            t = mk(gw, [128, 128], f"btd{kc}", bf16)
            nc.vector.tensor_copy(t[:, 0:64], btall[:, 64 * kc:64 * kc + 64])
            nc.vector.tensor_copy(t[:, 64:128], btall[:, 64 * kc:64 * kc + 64])
            btd.append(t)
        w["bt"] = btd
        c2w = load_conv(l, "c2w", 64, 32)[0]
        c2wf = mk(gw, [64, 32], "c2wf")
        nc.vector.tensor_copy(c2wf[:], c2w[:])
        w["c2w"] = c2wf
        w["w2cA"] = load_conv(l, "w2cA", 32, 128)[0]
        w["w2cB"] = load_conv(l, "w2cB", 128, 128)[0]
        g2c = load_conv(l, "g2c", 128, 128)[0]
        wAB = mk(gw, [128, 128], "wAB", f16)
        nc.vector.tensor_scalar(wAB[:], g2c[:], 0.125, None, ALU.mult)
        w["wAB"] = wAB
        w["cw"] = cw_tiles[l]
        return w

    def prefetch_p2(l, w):
        """Residual-block conv loads (need ccB)."""
        w["skip"] = load_conv(l, "skip", 256, 64)
        w["c1"] = load_conv(l, "c1", 256, 128)
        w["c2"] = load_conv(l, "c2", 128, 128)[0]
        w["c3"] = load_conv(l, "c3", 128, 64)[0]
        return w

    # ---------------- surf -> packed cos/sin tile (128 partitions)
    def emit_surf(l, w):
        hw = HWS[l]
        cs = 256 if l == 0 else 512        # spatial chunk width
        nch = hw // cs
        cossin = mk(big, [128, hw], "cossin", f16)
        for c in range(nch):
            if l == 2:
                tile, cl = wbt_tiles[l][c // 2], c % 2
            else:
                tile, cl = wbt_tiles[l][0], c
            ps = mk(psp, [128, 512], "we", bufs=2)
            for kc in range(8):
                j, i = kc // 2, kc % 2
                lo = cl * 8 * cs + j * 2 * cs + i * cs
                mmr(ps[:, :cs], w["bt"][kc][:], tile[:, lo:lo + cs],
                    kc == 0, kc == 7)
            tt = mk(ab, [128, 512], "trigtmp")
            nc.scalar.activation(tt[:, :cs], ps[:, :cs], ACTF.Sin,
                                 bias=w["bt_b2"][:], scale=SIN_SCALE)
            nc.vector.tensor_scalar(cossin[:, c * cs:(c + 1) * cs],
                                    tt[:, :cs], w["trig_b"][:], None,
                                    ALU.subtract)
        return cossin

    # ---------------- DVE rsqrt (quake + 2 newton steps)
    def rsqrt_dve(ve, nm):
        y = mk(smc, [128, 2], f"rsq{nm}")
        yi = y[:].bitcast(i32)
        nc.vector.tensor_scalar(yi, ve[:].bitcast(i32), 1, None,
                                ALU.arith_shift_right)
        nc.vector.tensor_scalar(yi, yi, RSQRT_MAGIC, None, ALU.subtract)
        nc.vector.tensor_scalar(yi, yi, -1, None, ALU.mult)
        t = mk(smc, [128, 2], f"rsqt{nm}")
        for _ in range(2):
            nc.vector.tensor_mul(t[:], y[:], y[:])
            nc.vector.tensor_mul(t[:], t[:], ve[:])
            nc.vector.tensor_scalar(t[:], t[:], -0.5, 1.5, ALU.mult, ALU.add)
            nc.vector.tensor_mul(y[:], y[:], t[:])
        return y

    # ---------------- emit everything
    w = prefetch_bias(0)
    phase_a_emit(0)
    phase_a_emit(1)
    phase_a_emit(2, defer=True)
    w = prefetch_p1(0, w)

    for l in range(L):
        s = SIZES[l]
        hw = HWS[l]
        nch = max(1, hw // 512)
        cn0 = min(512, hw)

        cossin = emit_surf(l, w)

        img_sb = None
        if l == L - 1:
            img_sb = mk(big, [3, hw], "img", f16)

        for call in range(2):
            do_drain = (l, call) >= (0, 1)
            # ---- sobel prep (f16 shifted adds)
            d3 = mk(big, [F, hw], "d3", f16)
            f3p = mk(big, [F, hw], "f3p", f16)
            o3 = mk(big, [F, hw], "o3", f16)
            U = mk(big, [128, hw], "upk", f16)
            bf = out_bf
            d33 = d3[:].rearrange("p (h w) -> p h w", h=s)
            f33 = f3p[:].rearrange("p (h w) -> p h w", h=s)
            bf3 = out_bf[:, 0:hw].rearrange("p (h w) -> p h w", h=s)
            nc.vector.tensor_sub(d3[:, 1:hw - 1], bf[:, 2:hw], bf[:, 0:hw - 2])
            nc.vector.tensor_copy(d33[:, :, 0:1], bf3[:, :, 1:2])
            nc.vector.tensor_scalar(d33[:, :, s - 1:s], bf3[:, :, s - 2:s - 1],
                                    -1.0, None, ALU.mult)
            nc.vector.tensor_add(f3p[:, 1:hw - 1], bf[:, 2:hw],
                                 bf[:, 0:hw - 2])
            nc.vector.tensor_copy(f33[:, :, 0:1], bf3[:, :, 1:2])
            nc.vector.tensor_copy(f33[:, :, s - 1:s], bf3[:, :, s - 2:s - 1])
            nc.vector.tensor_scalar(o3[:, :], bf[:, 0:hw], 2.0, None, ALU.mult)
            nc.vector.tensor_add(o3[:, :], o3[:, :], f3p[:, :])
            # wave coefficients
            so = mk(smc, [F, 1], "so")
            nc.vector.tensor_reduce(so[:], out_bf[:, 0:hw], axis=AX.X,
                                    op=ALU.add)
            ps_c = mk(psp, [CPE, 1], "we", bufs=2)
            mmr(ps_c[:], w["c2w"][:], so[:], True, True)
            coef = mk(smc, [CPE, 1], "coef")
            nc.scalar.activation(coef[:], ps_c[:], ACTF.Identity,
                                 bias=w["c2w_b"][:])
            w2cA_eff = mk(gw, [CPE, 128], "w2cAe", f16)
            nc.vector.tensor_scalar(w2cA_eff[:], w["w2cA"][:], coef[:], None,
                                    ALU.mult)

            def emit_U(a, b):
                ua, ub = max(a, s), min(b, hw - s)
                nc.vector.tensor_add(U[0:F, ua:ub], d3[:, ua - s:ub - s],
                                     d3[:, ua + s:ub + s])
                if a == 0:
                    nc.vector.tensor_copy(U[0:F, 0:s], d3[:, s:2 * s])
                if b == hw:
                    nc.vector.tensor_copy(U[0:F, hw - s:hw],
                                          d3[:, hw - 2 * s:hw - s])
                d2t = mk(big, [F, hw], "d2t", f16)
                nc.vector.tensor_scalar(d2t[:, a:b], d3[:, a:b], 2.0, None,
                                        ALU.mult)
                nc.vector.tensor_add(U[0:F, a:b], U[0:F, a:b], d2t[:, a:b])
                nc.vector.tensor_sub(U[F:128, ua:ub], o3[:, ua + s:ub + s],
                                     o3[:, ua - s:ub - s])
                if a == 0:
                    nc.vector.tensor_copy(U[F:128, 0:s], o3[:, s:2 * s])
                if b == hw:
                    nc.vector.tensor_scalar(U[F:128, hw - s:hw],
                                            o3[:, hw - 2 * s:hw - s], -1.0,
                                            None, ALU.mult)

            # ---- pass 1: ge/we raw matmuls + stats
            we_raw = mk(big, [128, hw], "weraw", f16)
            ge_raw = mk(big, [128, hw], "geraw", f16)
            we_st = mk(smc, [128, 6 * nch], "west")
            ge_st = mk(smc, [128, 6 * nch], "gest")
            for n in range(nch):
                c0 = n * cn0
                cn = cn0
                if n % 4 == 0:
                    emit_U(c0, min(c0 + 4 * cn0, hw))
                ps_ge = mk(psp, [128, 512], "g1", bufs=2)
                mmr(ps_ge[:, :cn], w["wAB"][:], U[:, c0:c0 + cn], True, True)
                ps_we = mk(psp, [128, 512], "we", bufs=2)
                mmr(ps_we[:, :cn], w2cA_eff[:], w["cw"][:, c0:c0 + cn],
                    True, False)
                mmr(ps_we[:, :cn], w["w2cB"][:], cossin[:, c0:c0 + cn],
                    False, True)
                nc.vector.bn_stats(ge_st[:, n * 6:(n + 1) * 6], ps_ge[:, :cn])
                nc.scalar.copy(ge_raw[:, c0:c0 + cn], ps_ge[:, :cn])
                nc.vector.bn_stats(we_st[:, n * 6:(n + 1) * 6], ps_we[:, :cn])
                nc.scalar.copy(we_raw[:, c0:c0 + cn], ps_we[:, :cn])
                if do_drain:
                    drain(1)

            if call == 0:
                w = prefetch_p2(l, w)

            # ---- fold instance norm into the skip/c1 weights
            mv = mk(smc, [128, 4], "mv")
            nc.vector.bn_aggr(mv[:, 0:2], ge_st[:])
            nc.vector.bn_aggr(mv[:, 2:4], we_st[:])
            ve = mk(smc, [128, 2], "ve")
            nc.vector.tensor_scalar(ve[:, 0:1], mv[:, 1:2], 1e-5, None,
                                    ALU.add)
            nc.vector.tensor_scalar(ve[:, 1:2], mv[:, 3:4], 1e-5, None,
                                    ALU.add)
            rs = rsqrt_dve(ve, "a")
            m_bf = mk(smc, [128, 2], "m_bf", f16)
            nc.vector.tensor_copy(m_bf[:, 0:1], mv[:, 0:1])
            nc.vector.tensor_copy(m_bf[:, 1:2], mv[:, 2:3])
            sk0s = mk(gw, [128, F], "sk0s", f16)
            nc.vector.tensor_scalar(sk0s[:], w["skip"][0][:], rs[:, 0:1],
                                    None, ALU.mult)
            sk1s = mk(gw, [128, F], "sk1s", f16)
            nc.vector.tensor_scalar(sk1s[:], w["skip"][1][:], rs[:, 1:2],
                                    None, ALU.mult)
            c10s = mk(gw, [128, 128], "c10s", f16)
            nc.vector.tensor_scalar(c10s[:], w["c1"][0][:], rs[:, 0:1],
                                    None, ALU.mult)
            c11s = mk(gw, [128, 128], "c11s", f16)
            nc.vector.tensor_scalar(c11s[:], w["c1"][1][:], rs[:, 1:2],
                                    None, ALU.mult)
            ps_f1 = mk(psp, [F, 1], "sc", bufs=2)
            mmr(ps_f1[:], sk0s[:], m_bf[:, 0:1], True, False)
            mmr(ps_f1[:], sk1s[:], m_bf[:, 1:2], False, True)
            bias_sc = mk(smc, [F, 1], "bias_sc")
            nc.vector.tensor_sub(bias_sc[:], w["sbc3"][:], ps_f1[:])
            nc.vector.tensor_mul(bias_sc[:], bias_sc[:], leak64[:])
            ps_f2 = mk(psp, [128, 1], "sc", bufs=2)
            mmr(ps_f2[:], c10s[:], m_bf[:, 0:1], True, False)
            mmr(ps_f2[:], c11s[:], m_bf[:, 1:2], False, True)
            c1_be = mk(smc, [128, 1], "c1_be")
            nc.vector.tensor_sub(c1_be[:], w["c1_b"][:], ps_f2[:])
            if do_drain:
                drain(1)

            # ---- pass 2: skip/c1 -> prelu -> c2 -> prelu -> c3, residual
            ps_s = [None] * nch
            ps_1 = [None] * nch

            def emit_front(n):
                c0 = n * cn0
                cn = cn0
                ps_s[n] = mk(psp, [F, 512], "sc", bufs=2)
                mmr(ps_s[n][:, :cn], sk0s[:], ge_raw[:, c0:c0 + cn],
                    True, False)
                mmr(ps_s[n][:, :cn], sk1s[:], we_raw[:, c0:c0 + cn],
                    False, False)
                ps_1[n] = mk(psp, [128, 512], "g1", bufs=2)
                mmr(ps_1[n][:, :cn], c10s[:], ge_raw[:, c0:c0 + cn],
                    True, False)
                mmr(ps_1[n][:, :cn], c11s[:], we_raw[:, c0:c0 + cn],
                    False, True)

            def lrelu_act(dst, src, bias):
                if SIM_COMPAT:
                    nc.scalar.activation(dst, src, ACTF.Identity, bias=bias)
                    nc.vector.scalar_tensor_tensor(dst, dst, 0.2, dst,
                                                   ALU.mult, ALU.max)
                else:
                    nc.scalar.activation(dst, src, ACTF.Prelu, bias=bias,
                                         alpha=0.2)

            def emit_back(n):
                c0 = n * cn0
                cn = cn0
                a1 = mk(ab, [128, 512], "a1", f16)
                lrelu_act(a1[:, :cn], ps_1[n][:, :cn], c1_be[:])
                ps_2 = mk(psp, [128, 512], "we", bufs=2)
                mmr(ps_2[:, :cn], w["c2"][:], a1[:, :cn], True, True)
                a2 = mk(ab, [128, 512], "a2", f16)
                lrelu_act(a2[:, :cn], ps_2[:, :cn], w["c2_b"][:])
                mmr(ps_s[n][:, :cn], w["c3"][:], a2[:, :cn], False, True)
                tn = mk(ab, [F, 512], "tn", f16)
                nc.scalar.activation(tn[:, :cn], ps_s[n][:, :cn],
                                     ACTF.Identity, bias=bias_sc[:],
                                     scale=leak64[:])
                if l == L - 1:
                    nc.gpsimd.tensor_add(out_bf[:, c0:c0 + cn],
                                         out_bf[:, c0:c0 + cn], tn[:, :cn])
                else:
                    nc.vector.tensor_add(out_bf[:, c0:c0 + cn],
                                         out_bf[:, c0:c0 + cn], tn[:, :cn])
                if l == L - 1 and call == 1:
                    ps_i = mk(psp, [3, 512], "g1", bufs=2)
                    mmr(ps_i[:, :cn], imgWT_bf[:], out_bf[:, c0:c0 + cn],
                        True, True)
                    nc.scalar.activation(img_sb[:, c0:c0 + cn], ps_i[:, :cn],
                                         ACTF.Identity, bias=imgb[:])
                    nc.vector.tensor_scalar(img_sb[:, c0:c0 + cn],
                                            img_sb[:, c0:c0 + cn], -1.0, 1.0,
                                            ALU.max, ALU.min)

            for n in range(nch + 1):
                if n < nch:
                    emit_front(n)
                if n >= 1:
                    emit_back(n - 1)
                    if do_drain:
                        drain(1)
            if do_drain:
                drain(2)

        # -------- layer transition: bilinear 2x upsample + gauss (f16)
        if l < L - 1:
            s2 = 2 * s
            o3v = out_bf[:, 0:hw].rearrange("p (h w) -> p h w", h=s)
            po = mk(big, [F, hw], "d3", f16)
            nc.vector.tensor_scalar(po[:, :], out_bf[:, 0:hw], 1.0 / 3.0, None,
                                    ALU.mult)
            po3 = po[:].rearrange("p (h w) -> p h w", h=s)
            upv = mk(big, [F, s2 * s], "f3p", f16)
            v3 = upv[:].rearrange("p (h w) -> p h w", h=s2)
            nc.vector.tensor_add(v3[:, 2:s2:2, :], o3v[:, 1:s, :],
                                 po3[:, 0:s - 1, :])
            nc.vector.tensor_add(v3[:, 1:s2 - 1:2, :], o3v[:, 0:s - 1, :],
                                 po3[:, 1:s, :])
            nc.vector.tensor_scalar(v3[:, 0:1, :], o3v[:, 0:1, :], 4.0 / 3.0,
                                    None, ALU.mult)
            nc.vector.tensor_scalar(v3[:, s2 - 1:s2, :], o3v[:, s - 1:s, :],
                                    4.0 / 3.0, None, ALU.mult)
            ph = mk(big, [F, s2 * s], "o3", f16)
            nc.vector.tensor_scalar(ph[:, :s2 * s], upv[:, :s2 * s],
                                    1.0 / 3.0, None, ALU.mult)
            ph3 = ph[:].rearrange("p (h w) -> p h w", h=s2)
            uph = mk(big, [F, s2 * s2], "weraw", f16)
            h3 = uph[:].rearrange("p (h w) -> p h w", h=s2)
            nc.vector.tensor_add(h3[:, :, 2:s2:2], v3[:, :, 1:s],
                                 ph3[:, :, 0:s - 1])
            nc.vector.tensor_add(h3[:, :, 1:s2 - 1:2], v3[:, :, 0:s - 1],
                                 ph3[:, :, 1:s])
            nc.vector.tensor_scalar(h3[:, :, 0:1], v3[:, :, 0:1], 4.0 / 3.0,
                                    None, ALU.mult)
            nc.vector.tensor_scalar(h3[:, :, s2 - 1:s2], v3[:, :, s - 1:s],
                                    4.0 / 3.0, None, ALU.mult)
            hw2 = s2 * s2
            gu = mk(big, [F, hw2], "geraw", f16)
            nc.vector.tensor_add(gu[:, s2:hw2 - s2], uph[:, 0:hw2 - 2 * s2],
                                 uph[:, 2 * s2:hw2])
            nc.vector.tensor_copy(gu[:, 0:s2], uph[:, s2:2 * s2])
            nc.vector.tensor_copy(gu[:, hw2 - s2:hw2],
                                  uph[:, hw2 - 2 * s2:hw2 - s2])
            hg = mk(big, [F, hw2], "d2t", f16)
            nc.vector.tensor_scalar(hg[:, :], uph[:, :], GRBA, None, ALU.mult)
            nc.vector.tensor_add(gu[:, :], gu[:, :], hg[:, :])
            gu3 = gu[:].rearrange("p (h w) -> p h w", h=s2)
            gu2 = mk(big, [F, hw2], "upk", f16)
            g23 = gu2[:].rearrange("p (h w) -> p h w", h=s2)
            nc.vector.tensor_add(g23[:, :, 1:s2 - 1], gu3[:, :, 0:s2 - 2],
                                 gu3[:, :, 2:s2])
            nc.vector.tensor_copy(g23[:, :, 0:1], gu3[:, :, 1:2])
            nc.vector.tensor_copy(g23[:, :, s2 - 1:s2], gu3[:, :, s2 - 2:s2 - 1])
            hg2 = mk(big, [F, hw2], "cossin", f16)
            nc.vector.tensor_scalar(hg2[:, :], gu[:, :], GRBA, None, ALU.mult)
            nc.vector.tensor_add(gu2[:, :], gu2[:, :], hg2[:, :])
            newout = mk(big, [F, HWS[-1]], "obf", f16)
            nc.vector.tensor_scalar(newout[:, :hw2], gu2[:, :], UPGAUSS_C,
                                    None, ALU.mult)
            out_bf = newout
            # all pending phase-A groups must be emitted before prefetch(2):
            # its scalar-queue DMAs wait on ccA2/ccB2, which need those
            # groups' (scalar-queue) stage copies — emitting them after
            # would deadlock the scalar queue.
            drain(3 if l == 0 else 99)
            w = prefetch_bias(l + 1)
            w = prefetch_p1(l + 1, w)
        else:
            nc.gpsimd.dma_start(out_img[:, :], img_sb[:])

    for p in reversed(ctxs):
        p.__exit__(None, None, None)


_MODULE_CACHE = {}


def build_module():
    if "nc" in _MODULE_CACHE:
        return _MODULE_CACHE["nc"]
    from concourse import bacc, mybir
    import concourse.tile as tile

    nc = bacc.Bacc("TRN2", target_bir_lowering=False, debug=False,
                   num_devices=NCORE)
    f32 = mybir.dt.float32
    dts = {"f32": mybir.dt.float32, "bf16": mybir.dt.bfloat16,
           "f16": mybir.dt.float16, "f8": mybir.dt.float8e4}
    ins = {}
    for name, shape, dt in INPUT_SHAPES:
        ins[name] = nc.dram_tensor(name, list(shape), dts[dt],
                                   kind="ExternalInput").ap()
    out_img = nc.dram_tensor("img", [3, HWS[-1]], mybir.dt.float16,
                             kind="ExternalOutput").ap()
    with tile.TileContext(nc) as tc:
        build_kernel(tc, ins, out_img)
    nc.compile()
    _MODULE_CACHE["nc"] = nc
    return nc


# ---------------------------------------------------------------- host prep
def host_prep(inputs):
    """Full (unsharded) numpy inputs -> per-core in_maps."""
    import ml_dtypes

    bf16 = ml_dtypes.bfloat16
    fp8 = ml_dtypes.float8_e4m3

    lat = np.asarray(inputs["lat"], np.float32)
    leak = float(np.clip(np.asarray(inputs["leak_factor"]), 0.001, 1000.0))
    seed = np.asarray(inputs["seed"], np.float32)[0].reshape(F, 256)
    frac_W = np.asarray(inputs["frac_W"], np.float32)
    frac_b = np.asarray(inputs["frac_b"], np.float32)

    def T(W, fin, fout):
        W = np.asarray(W, np.float32)
        return W.reshape(L, LAT, fout, fin).transpose(0, 1, 3, 2)

    bt_T = T(inputs["bt_W"], 1024, F)
    c2w_T = T(inputs["c2w_W"], F, CPE).copy()
    for l in range(L):
        c2w_T[l] *= 1.0 / HWS[l]
    w2c_T = T(inputs["w2c_W"], CPE + 2 * F, 2 * F)
    convT = {
        "c2w": c2w_T,
        "w2cA": w2c_T[:, :, :CPE, :],
        "w2cB": np.ascontiguousarray(w2c_T[:, :, CPE:, :]) * np.float32(E1),
        "g2c": T(inputs["g2c_W"], 2 * F, 2 * F),
        "skip": T(inputs["cc_skip_W"], 4 * F, F),
        "c1": T(inputs["cc_c1_W"], 4 * F, 2 * F),
        "c2": T(inputs["cc_c2_W"], 2 * F, 2 * F),
        "c3": T(inputs["cc_c3_W"], 2 * F, F),
    }
    # lat_new per layer/sample (tiny host matmul — input prep), doubled cols
    lat_new = np.stack([lat @ frac_W[l] + frac_b[l] for l in range(L)])
    latn = np.concatenate([lat_new.transpose(0, 2, 1)] * 2, axis=2)
    latn = np.ascontiguousarray(latn)          # (L, LAT, 2B)
    bias_blk = np.concatenate(
        [np.asarray(inputs[k], np.float32) for _n, k, _fo in BIAS_KEYS],
        axis=2)                                # (L, LAT, 480)

    common = {
        "latn": latn,
        "biasW": bias_blk.astype(bf16),
        "leak": np.full((1, 1), leak, np.float32),
        "seed": np.ascontiguousarray(seed),
        "imgWT": np.ascontiguousarray(np.asarray(inputs["img_W"],
                                                 np.float32).T),
        "imgb": np.asarray(inputs["img_b"], np.float32).reshape(3, 1),
    }
    for l in range(L):
        hw = HWS[l]
        cs = 256 if l == 0 else 512
        wbf = np.asarray(inputs[f"wave_bias{l}"], np.float32)[0].reshape(
            F * 16, hw)
        # (128, nch, jpair, slot, cs): DoubleRow rhs layout
        packed = wbf.reshape(4, 2, 128, hw // cs, cs).transpose(
            2, 3, 0, 1, 4).reshape(128, 8 * hw)
        common[f"wb{l}"] = np.ascontiguousarray(packed * S_WB).astype(fp8)
        common[f"cw{l}"] = pos_enc_np(SIZES[l]).astype(np.float16)

    in_maps = []
    for r in range(NCORE):
        m = dict(common)
        m["latnme"] = np.ascontiguousarray(
            lat_new[:, r % B, :][:, :, None])   # (L, LAT, 1)
        w8 = bt_T[:, :, r * 128:(r + 1) * 128, :].reshape(L, LAT, BT_W)
        # DoubleRow rhs layout: (128, chunk, slot, 512), slot = LAT half
        w8 = w8.reshape(L, 2, 128, BT_W // 512, 512).transpose(
            0, 2, 3, 1, 4).reshape(L, 128, 2 * BT_W)
        m["wws8"] = np.ascontiguousarray(w8 * S_BT).astype(fp8)
        wsb = np.zeros((L, LAT, WSB_W), np.float32)
        for n, fi, fo in P1_CONVS + P2_CONVS:
            rows = min(fi, 128)
            do = fo // NCORE
            for kc in range(fi // rows):
                off = WSB_OFF[(n, kc)]
                wsb[:, :, off:off + rows * do] = convT[n][
                    :, :, kc * rows:(kc + 1) * rows,
                    r * do:(r + 1) * do].reshape(L, LAT, rows * do)
        m["wwsb"] = wsb.astype(bf16)
        in_maps.append(m)
    return in_maps


def kernel(**inputs):
    from concourse.bass_utils import run_bass_kernel_spmd

    nc = build_module()
    in_maps = host_prep(inputs)
    res = run_bass_kernel_spmd(nc, in_maps, list(range(NCORE)))
    imgs = [res.results[b]["img"].reshape(3, SIZES[-1], SIZES[-1])
            for b in range(B)]
    return np.stack(imgs).astype(np.float32)


if __name__ == "__main__":
    nc = build_module()
    print("module built ok; instructions:",
          sum(len(bb.instructions) for bb in nc.main_func.blocks))
